# revision 69
# baseline (speedup 1.0000x reference)
"""EquivariantAttention Trainium2 kernel.

B=2, L=2048, D=512, H=8, HD=64 over 8 NeuronCores.
Head-parallel attention (core c owns head c, both batches), AllToAll to
sequence-shard the output projection (core c owns q-window [256c, 256c+256)).

Math notes:
  Qi . Ki = ||Q||*||K|| + (Bq Q) . (Bk K), Bq/Bk = basis[:63] rows.
  -> 64-row operands: qS = [Bq Q ; ||Q||], kS = [Bk K ; ||K|| - muk]
  (muk centering is softmax-invariant: the -muk*||Q|| term is constant
  along k). Scores are computed transposed ([k, q]); the softmax
  denominator comes from an appended ones-row in V (row 64).
  Softmax is max-free (scores bounded for this problem's scale).

Layout/engine strategy (cost-model driven):
  - x, weights, V, exp(scores) in bf16 (same PE rate, half DMA/SBUF).
  - V computed directly transposed: lhsT = x tile, rhs = Wv^T (N=64).
  - qS/kS ip rows via one block-diag matmul per 512-slice; both halves
    live in one [128, L] tile (kS at partitions 64..127; scores matmuls
    use explicit tile_position=(0,0)).
  - sum-of-squares via one block-ones matmul; one ACT sqrt writes both
    norm rows (partition-strided AP); squares + muk-sub on GPSIMD.
  - exp merged per k-tile pair ([128, 2, 512] PSUM AP); causal-trimmed
    matmuls; stale PSUM columns are exp'd but never consumed.
  - per-batch AllToAll (bf16); outproj feeds recv bf16 straight into
    the PE; merged DMAs throughout.
"""

import sys

sys.path.insert(0, "/opt/trn_rl_repo")

import numpy as np

import concourse.bass as bass  # noqa: F401  (AP helpers)
import concourse.tile as tile
from concourse import bacc, mybir
from concourse.bass_utils import run_bass_kernel_spmd

F32 = mybir.dt.float32
F32R = mybir.dt.float32r
BF16 = mybir.dt.bfloat16
F16 = mybir.dt.float16
TABLE_PATCH = False
EXP = mybir.ActivationFunctionType.Exp
SQRT = mybir.ActivationFunctionType.Sqrt

B, L, D, H, HD = 2, 2048, 512, 8, 64
NC = 8
LW = L // NC          # 256: per-core q-window for the output projection
NL = 4                # l-slices of 512 per batch
NK = L // 128         # 16 k-tiles per batch
NW = 4                # q-windows of 512 per batch


def _build_causal():
    # Restrict the ACT table chooser to the one set holding every
    # function this kernel uses (Exp, Ln, Copy, Identity): the greedy
    # per-transition chooser otherwise ping-pongs between the exp-only
    # and ln-only sets, costing a 1.3us table load each time.
    import concourse.bacc as _bacc_mod
    _orig_tables = _bacc_mod.get_activation_tables
    if TABLE_PATCH:
        _bacc_mod.get_activation_tables = lambda arch: {
            "natural_log_exp_and_others":
                _orig_tables(arch)["natural_log_exp_and_others"]}
    try:
        return _build_causal_inner()
    finally:
        _bacc_mod.get_activation_tables = _orig_tables


def _build_causal_inner():
    nc = bacc.Bacc("TRN2", target_bir_lowering=False, debug=False,
                   enable_asserts=True, num_devices=NC)

    xt = nc.dram_tensor("xt", [B, D, L], F16, kind="ExternalInput")
    wqk4 = nc.dram_tensor("wqk4", [128, 512], F16, kind="ExternalInput")
    wv4 = nc.dram_tensor("wv4", [128, 256], F16, kind="ExternalInput")
    wo4 = nc.dram_tensor("wo4", [128, 2048], F16, kind="ExternalInput")
    bdm = nc.dram_tensor("bdm", [128, 128], F32R, kind="ExternalInput")
    obm = nc.dram_tensor("obm", [128, 2], F16, kind="ExternalInput")
    bqk = nc.dram_tensor("bqk", [128, 1], F32, kind="ExternalInput")
    bv = nc.dram_tensor("bv", [128, 8 * HD], F32, kind="ExternalInput")
    bo4 = nc.dram_tensor("bo4", [128, 4], F32, kind="ExternalInput")
    muk2 = nc.dram_tensor("muk2", [2, 1], F32, kind="ExternalInput")
    onr = nc.dram_tensor("onr", [1, HD], F32R, kind="ExternalInput")
    tri2 = nc.dram_tensor("tri2", [128, 128], BF16, kind="ExternalInput")
    yts = nc.dram_tensor("yts", [B, D, LW], F32, kind="ExternalOutput")

    from contextlib import ExitStack
    with tile.TileContext(nc) as tc, ExitStack() as ctx:
        ec = ctx.enter_context
        const = ec(tc.tile_pool(name="const", bufs=1))
        xtp = ec(tc.tile_pool(name="xtp", bufs=8))
        qkrp = ec(tc.tile_pool(name="qkrp", bufs=1))
        qkp = ec(tc.tile_pool(name="qkp", bufs=2))
        ksp = ec(tc.tile_pool(name="ksp", bufs=2))
        sqp = ec(tc.tile_pool(name="sqp", bufs=2))
        vtp = ec(tc.tile_pool(name="vtp", bufs=2))
        expp = ec(tc.tile_pool(name="expp", bufs=3))
        uscp = ec(tc.tile_pool(name="uscp", bufs=2))
        rzp = ec(tc.tile_pool(name="rzp", bufs=1))
        zbp = ec(tc.tile_pool(name="zbp", bufs=1))
        nmp = ec(tc.tile_pool(name="nmp", bufs=2))
        rvp = ec(tc.tile_pool(name="rvp", bufs=1))
        ytp = ec(tc.tile_pool(name="ytp", bufs=1))
        dumb = ec(tc.tile_pool(name="dumb", bufs=1))
        pp = ec(tc.tile_pool(name="pp", bufs=2, space="PSUM"))      # 2 banks
        vp8 = ec(tc.tile_pool(name="vp8", bufs=1, space="PSUM"))    # 1 bank
        up = ec(tc.tile_pool(name="up", bufs=1, space="PSUM"))      # 1 bank
        sp = ec(tc.tile_pool(name="sp", bufs=2, space="PSUM"))      # 2x2 banks
        dram = ec(tc.tile_pool(name="dram", bufs=1, space="DRAM"))

        # ---- constants: sync queue feeds the projection path (and x),
        # scalar/vector queues take the rest; gpsimd stays free for compute
        wqk_sb = const.tile([128, 4, 128], F16)
        wv_sb = const.tile([128, 4, HD], F16)
        wo_sb = const.tile([128, 4, D], F16)
        bd_sb = const.tile([128, 128], F32R)
        ob_sb = const.tile([128, 2], F16)
        bqk_sb = const.tile([128, 1], F32)
        bv_sb = const.tile([128, 8, HD], F32)
        bo_sb = const.tile([128, 4], F32)
        muk2_sb = const.tile([2, 1], F32)
        onr_sb = const.tile([1, HD], F32R)
        tri_sb = const.tile([128, 128], BF16)
        shift_sb = const.tile([128, 1], F32)  # softmax global shift
        dum_sb = dumb.tile([128, 512], F16)  # PE warmup operand

        with tc.high_priority():
            nc.sync.dma_start(out=wqk_sb[:, :, :],
                              in_=wqk4[:, :].rearrange("p (c m) -> p c m",
                                                       c=4))
            nc.scalar.dma_start(out=wv_sb[:, :, :],
                                in_=wv4[:, :].rearrange("p (c m) -> p c m",
                                                        c=4))
        nc.vector.memset(shift_sb[:, :], -20.0)
        nc.vector.memset(dum_sb[:, :], 0.125)

        def consts_early():
            # issued behind the batch-0 x tiles on HWDGE
            nc.scalar.dma_start(out=bqk_sb[:, :], in_=bqk[:, :])
            nc.scalar.dma_start(out=bd_sb[:, :], in_=bdm[:, :])
            nc.scalar.dma_start(out=ob_sb[:, :], in_=obm[:, :])
            nc.scalar.dma_start(out=muk2_sb[:, :], in_=muk2[:, :])
            nc.scalar.dma_start(out=onr_sb[:, :], in_=onr[:, :])
            nc.scalar.dma_start(out=bv_sb[:, :, :],
                                in_=bv[:, :].rearrange("p (j m) -> p j m", j=8))
            nc.gpsimd.dma_start(out=tri_sb[:, :], in_=tri2[:, :])

        def consts_late():
            nc.gpsimd.dma_start(out=wo_sb[:, :, :],
                                in_=wo4[:, :].rearrange("p (c m) -> p c m",
                                                        c=4))
            nc.gpsimd.dma_start(out=bo_sb[:, :], in_=bo4[:, :])

        # ---- PE pstate warmup: keep the array busy until x arrives ----
        dum_ps = sp.tile([128, 2, 512], F32, tag="sp")
        for _ in range(6):
            nc.tensor.matmul(dum_ps[:, 0, :], dum_sb[:, 0:128], dum_sb[:, :],
                             start=True, stop=True)

        last_send = {}
        last_exp = {}
        last_sqrt = {}
        send = [dram.tile([NC, HD, LW], F16, tag=f"send{b}", name=f"send{b}")
                for b in range(B)]
        recv = [dram.tile([NC, HD, LW], F16, tag=f"recv{b}", name=f"recv{b}")
                for b in range(B)]
        jobs = {}

        def phase_p(b):
            sqrt_ops = []
            ssq_ops = []
            # x for this batch: one DMA per 128-feature chunk.
            from contextlib import nullcontext
            xth = [[None] * 2 for _ in range(4)]
            with tc.high_priority() if b == 0 else nullcontext():
                for h in range(2):
                    for dc in range(4):
                        t = xtp.tile([128, L // 2], F16, tag=f"xts{dc}h{h}")
                        nc.sync.dma_start(
                            out=t[:, :],
                            in_=xt[b, 128 * dc:128 * (dc + 1),
                                   1024 * h:1024 * (h + 1)])
                        xth[dc][h] = t
            qkr = qkrp.tile([128, L], F32R, tag="qkr")   # raw Q;K (biased)

            qk = qkp.tile([64, L], F16, tag="qk")        # qS invariants
            ks = ksp.tile([64, L], F16, tag="ks")        # kS invariants
            sq = sqp.tile([128, L], F16, tag="sq")       # squares
            ssqs = []
            for ls in range(NL):
                s = slice(512 * ls, 512 * (ls + 1))
                qk_ps = pp.tile([128, 512], F32, tag="pp")
                for dc in range(4):
                    nc.tensor.matmul(qk_ps[:, :], wqk_sb[:, dc, :],
                                     xth[dc][ls // 2][:, 512 * (ls % 2):
                                                      512 * (ls % 2 + 1)],
                                     start=(dc == 0), stop=(dc == 3))
                nc.vector.tensor_scalar_add(qkr[:, s], qk_ps[:, :],
                                            bqk_sb[:, 0:1])
                ip_ps = pp.tile([128, 512], F32, tag="pp")
                nc.tensor.matmul(ip_ps[:, :], bd_sb[:, :], qkr[:, s],
                                 start=True, stop=True)
                with nc.allow_low_precision(reason="f16 squares"):
                    nc.gpsimd.tensor_mul(sq[:, s], qkr[:, s], qkr[:, s])
                with nc.allow_low_precision(reason="f16 invariants"):
                    nc.vector.tensor_copy(qk[0:63, s], ip_ps[0:63, :])
                    nc.vector.tensor_copy(ks[0:63, s], ip_ps[64:127, :])
                ssq_ps = pp.tile([128, 512], F32, tag="pp")
                nc.tensor.matmul(ssq_ps[0:2, :], ob_sb[:, :], sq[:, s],
                                 start=True, stop=True)
                nm2 = nmp.tile([2, 512], F16, tag="nm2")
                with nc.allow_low_precision(reason="f16 norms"):
                    sq_i = nc.scalar.activation(nm2[:, :], ssq_ps[0:2, :],
                                                SQRT)
                last_sqrt[b] = sq_i
                nc.gpsimd.tensor_scalar_sub(nm2[:, :], nm2[:, :],
                                            muk2_sb[:, 0:1])
                nc.sync.dma_start(out=qk[63:64, s], in_=nm2[0:1, :])
                nc.sync.dma_start(out=ks[63:64, s], in_=nm2[1:2, :])
            jobs[b] = (qk, ks, xth)

        def phase_v(b):
            qk, ks, xth = jobs[b]
            vt = vtp.tile([128, NK, HD + 1], BF16, tag="vt")
            for kt in range(NK):
                j = kt % 8
                if j == 0:
                    vt8 = vp8.tile([128, 8, HD], F32, tag="vp8")
                for dc in range(4):
                    nc.tensor.matmul(
                        vt8[:, j, :],
                        xth[dc][kt // 8][:, 128 * (kt % 8):
                                         128 * (kt % 8 + 1)],
                        wv_sb[:, dc, :], start=(dc == 0), stop=(dc == 3))
                if j == 7:
                    h8 = slice(kt - 7, kt + 1)
                    with nc.allow_low_precision(reason="bf16 V"):
                        nc.vector.tensor_add(vt[:, h8, 0:HD],
                                             vt8[:, :, :], bv_sb[:, :, :])
            with nc.allow_low_precision(reason="ones column"):
                nc.vector.memset(vt[:, :, HD:HD + 1], 1.0)
            jobs[b] = (qk, ks, vt)

        def attention(b):
            qk, ks, vt = jobs[b]
            pend = None     # deferred U-accumulation for the previous pair
            fin = None      # deferred normalization for the previous window

            def emit_u(item):
                u_ps, n, p, ex, los = item
                npair = 2 * (n + 1)
                for j in range(2):
                    ki = 2 * p + j
                    w = slice(los[j], 512)
                    nc.tensor.matmul(u_ps[:, w], vt[:, ki, :], ex[:, j, w],
                                     start=(p == 0 and j == 0),
                                     stop=(p == npair - 1 and j == 1))

            def emit_fin(item):
                u_ps, n = item
                rz = rzp.tile([1, 512], F32R, tag="rz")
                with nc.allow_low_precision(reason="f32r softmax denom"):
                    nc.vector.reciprocal(rz[:, :], u_ps[HD:HD + 1, :])
                zbb = zbp.tile([HD, 512], F32R, tag="zbb")
                nc.gpsimd.partition_broadcast(zbb[:, :], rz[:, :])
                usc = uscp.tile([HD, 512], F16, tag="usc")
                with nc.allow_low_precision(reason="bf16 payload"):
                    nc.vector.tensor_mul(usc[:, :], u_ps[0:HD, :],
                                         zbb[:, :])
                snd = nc.sync.dma_start(
                    out=send[b][2 * n:2 * n + 2, :, :].rearrange(
                        "h p c -> p h c"),
                    in_=usc[:, :].rearrange("p (h c) -> p h c", h=2))
                last_send[b] = snd

            for n in range(NW):
                qs = slice(512 * n, 512 * (n + 1))
                u_ps = up.tile([HD + 1, 512], F32, tag="up")
                for p in range(2 * (n + 1)):
                    st = sp.tile([128, 2, 512], F32, tag="sp")
                    los = []
                    for j in range(2):
                        ki = 2 * p + j
                        lo = max(0, 128 * (ki - 4 * n))
                        los.append(lo)
                        w = slice(lo, 512)
                        nc.tensor.matmul(
                            st[:, j, w],
                            ks[:, 128 * ki:128 * (ki + 1)],
                            qk[:, qs][:, w],
                            start=True, stop=True)
                    ex = expp.tile([128, 2, 512], BF16, tag="ex")
                    with nc.allow_low_precision(reason="bf16 softmax"):
                        if los[0] == los[1]:
                            e_i = nc.scalar.activation(ex[:, :, los[0]:512],
                                                       st[:, :, los[0]:512],
                                                       EXP, scale=0.125,
                                                       bias=shift_sb[:, 0:1])
                            if n == 0 and p == 0:
                                # start the exp stream only after this
                                # batch's sqrts: each sqrt<->exp interleave
                                # costs a 1.3us ACT table reload
                                e_i.ins.add_dependency(
                                    last_sqrt[b].ins.name,
                                    mybir.DependencyInfo.SYNC_ONLY)
                            last_exp[b] = e_i
                        else:
                            # exact-coverage split (no stale PSUM reads)
                            nc.scalar.activation(ex[:, :, los[1]:512],
                                                 st[:, :, los[1]:512], EXP,
                                                 scale=0.125,
                                                 bias=shift_sb[:, 0:1])
                            nc.scalar.activation(
                                ex[:, 0, los[0]:los[1]],
                                st[:, 0, los[0]:los[1]], EXP,
                                scale=0.125, bias=shift_sb[:, 0:1])
                    # causal triangle: zero the upper half post-exp (bf16
                    # all-SBUF multiply runs at 4x and off the ACT path)
                    for j in range(2):
                        ki = 2 * p + j
                        if ki >= 4 * n:
                            d = slice(los[j], los[j] + 128)
                            with nc.allow_low_precision(reason="bf16 mask"):
                                nc.vector.tensor_mul(ex[:, j, d], ex[:, j, d],
                                                     tri_sb[:, :])
                    if pend is not None:
                        emit_u(pend)
                    if fin is not None:
                        emit_fin(fin)
                        fin = None
                    pend = (u_ps, n, p, ex, los)
                fin = (u_ps, n)
            emit_u(pend)
            pend = None
            emit_fin(fin)
            fin = None

        def a2a(b):
            nc.gpsimd.collective_compute(
                "AllToAll", mybir.AluOpType.bypass,
                replica_groups=[list(range(NC))],
                ins=[send[b].opt()], outs=[recv[b].opt()],
            )

        def outproj(b):
            # keep collective-gated DMAs off the ACT queue: they would
            # head-of-line-block the other batch's exp stream
            rvh = rvp.tile([128, 4, LW], F16, tag="rvh")
            for dc in range(4):
                q = nc.gpsimd if b == 0 else (nc.sync if dc % 2 == 0
                                              else nc.scalar)
                d = q.dma_start(
                    out=rvh[:, dc, :],
                    in_=recv[b][2 * dc:2 * dc + 2, :, :].rearrange(
                        "j h c -> (j h) c"))
                if b == 0:
                    # schedule batch-0's output projection strictly after
                    # batch-1's attention: the scheduling sim under-predicts
                    # A(1)'s span and would otherwise pin these
                    # collective-gated loads ahead of ready A(1) work,
                    # head-of-line-blocking the PE stream
                    d.ins.add_dependency(last_send[1].ins.name,
                                         mybir.DependencyInfo.SYNC_ONLY)
            yt = ytp.tile([128, 4, LW], F32, tag="yt")
            for dp in range(2):
                y_ps = pp.tile([128, 512], F32, tag="pp")
                for dt_ in range(2):
                    dt = 2 * dp + dt_
                    for dc in range(4):
                        nc.tensor.matmul(
                            y_ps[:, 256 * dt_:256 * (dt_ + 1)],
                            wo_sb[:, dc, 128 * dt:128 * (dt + 1)],
                            rvh[:, dc, :], start=(dc == 0), stop=(dc == 3))
                for dt_ in range(2):
                    dt = 2 * dp + dt_
                    nc.vector.tensor_scalar_add(
                        yt[:, dt, :], y_ps[:, 256 * dt_:256 * (dt_ + 1)],
                        bo_sb[:, dt:dt + 1])
            q = nc.gpsimd if b == 0 else nc.sync
            for dp in range(2):
                q.dma_start(
                    out=yts[b, 256 * dp:256 * (dp + 1), :].rearrange(
                        "(d p) c -> p d c", p=128),
                    in_=yt[:, 2 * dp:2 * dp + 2, :])

        def dummies(k, gate=None):
            d_ps = vp8.tile([128, 8, HD], F32, tag="vp8")
            for i in range(k):
                m = nc.tensor.matmul(
                    d_ps[:, 0:8, :].rearrange("p a b -> p (a b)"),
                    dum_sb[:, 0:128], dum_sb[:, :], start=True, stop=True)
                if gate is not None and i == 0:
                    m.ins.add_dependency(gate.ins.name,
                                         mybir.DependencyInfo.SYNC_ONLY)

        consts_early()
        phase_p(0)
        phase_v(0)
        attention(0)
        consts_late()
        phase_p(1)
        phase_v(1)
        a2a(0)
        attention(1)
        a2a(1)
        outproj(0)
        outproj(1)
        dummies(24, gate=last_send[1])
    nc.compile()
    return nc


_CACHE = {}


def _get(causal: bool):
    assert causal
    if causal not in _CACHE:
        _CACHE[causal] = _build_causal()
    return _CACHE[causal]


def _make_w(coef):
    iu = np.triu_indices(D, k=1)
    a = np.zeros((D, D), np.float32)
    a[iu] = coef
    return a - a.T + np.eye(D, dtype=np.float32)


def _prep(x, mask, coef_q, coef_k, coef_v, coef_o,
          bias_q, bias_k, bias_v, bias_o, basis_q, basis_k):
    x = np.asarray(x, np.float32)
    mask = np.asarray(mask, np.float32)
    wq, wk, wv, wo = (_make_w(np.asarray(c, np.float32))
                      for c in (coef_q, coef_k, coef_v, coef_o))
    basis_q = np.asarray(basis_q, np.float32)
    basis_k = np.asarray(basis_k, np.float32)
    bq = np.asarray(bias_q, np.float32)
    bk = np.asarray(bias_k, np.float32)
    xtn = np.ascontiguousarray(x.transpose(0, 2, 1))
    wot = np.ascontiguousarray(wo.T)

    # causal fast path: mask[q, k] == 0 for k <= q else -1e9
    ii = np.arange(L)
    causal_ref = np.where(ii[None, :] <= ii[:, None], 0.0, -1e9).astype(np.float32)
    causal = bool(np.array_equal(mask, causal_ref))
    if not causal:
        return False, None

    bf16 = mybir.dt.np(mybir.dt.bfloat16)
    # block-diag ip lhsT: out rows 0..62 = Bq Q, 64..126 = Bk K
    bd = np.zeros((128, 128), np.float32)
    bd[0:HD, 0:HD - 1] = basis_q[:HD - 1, :].T
    bd[HD:128, HD:128 - 1] = basis_k[:HD - 1, :].T
    ob = np.zeros((128, 2), np.float32)
    ob[0:HD, 0] = 1.0
    ob[HD:128, 1] = 1.0
    # causal triangle for a diagonal 128-block ([k, q]: k > q masked),
    # pre-scaled by 8 (exp applies scale=1/8)
    kk = np.arange(128)
    tri2 = np.where(kk[:, None] <= kk[None, :], 1.0, 0.0).astype(np.float32)

    shared = {
        "xt": xtn.astype(np.float16), "bdm": bd,
        "obm": ob.astype(np.float16),
        "tri2": tri2.astype(bf16),
        "wo4": np.ascontiguousarray(
            wot.reshape(4, 128, D).transpose(1, 0, 2).reshape(128, 2048)
            ).astype(np.float16),
        "bo4": np.ascontiguousarray(
            np.asarray(bias_o, np.float32).reshape(4, 128).T),
        "onr": np.ones((1, HD), np.float32),
    }

    in_maps = []
    for c in range(NC):
        hs = slice(HD * c, HD * (c + 1))
        m = dict(shared)
        wqkt = np.concatenate([wq[hs, :].T, wk[hs, :].T], axis=1)   # [512, 128]
        m["wqk4"] = np.ascontiguousarray(
            wqkt.reshape(4, 128, 128).transpose(1, 0, 2).reshape(
                128, 512)).astype(np.float16)
        wvt = wv[hs, :].T                                            # [512, 64]
        m["wv4"] = np.ascontiguousarray(
            wvt.reshape(4, 128, HD).transpose(1, 0, 2).reshape(
                128, 256)).astype(np.float16)
        m["bqk"] = np.ascontiguousarray(
            np.concatenate([bq[hs], bk[hs]])[:, None])
        m["bv"] = np.ascontiguousarray(
            np.broadcast_to(np.asarray(bias_v, np.float32)[hs][None, None, :],
                            (128, 8, HD)).reshape(128, 8 * HD))
        m["muk2"] = np.array([[0.0], [np.linalg.norm(wk[hs, :])]],
                             np.float32)
        in_maps.append(m)
    return True, in_maps


def _kernel_numpy(x, mask, coef_q, coef_k, coef_v, coef_o,
                  bias_q, bias_k, bias_v, bias_o, basis_q, basis_k):
    x = np.asarray(x, np.float64)
    wq, wk, wv, wo = (_make_w(np.asarray(c, np.float32)).astype(np.float64)
                      for c in (coef_q, coef_k, coef_v, coef_o))
    def proj(t, w, b):
        return t @ w.T + np.asarray(b, np.float64)
    def split(t):
        return t.reshape(B, L, H, HD).transpose(0, 2, 1, 3)
    Q = split(proj(x, wq, bias_q))
    Kk = split(proj(x, wk, bias_k))
    V = split(proj(x, wv, bias_v))
    def inv(t, basis):
        nrm = np.linalg.norm(t, axis=-1, keepdims=True)
        ip = np.einsum('bhld,nd->bhln', t, np.asarray(basis, np.float64))
        return np.concatenate([nrm, ip], axis=-1)[..., :HD]
    Qi = inv(Q, basis_q)
    Ki = inv(Kk, basis_k)
    s = np.einsum('bhld,bhmd->bhlm', Qi, Ki) / np.sqrt(HD) + \
        np.asarray(mask, np.float64)
    s = s - s.max(axis=-1, keepdims=True)
    p = np.exp(s)
    p /= p.sum(axis=-1, keepdims=True)
    out = np.einsum('bhlm,bhmd->bhld', p, V)
    out = out.transpose(0, 2, 1, 3).reshape(B, L, D)
    return proj(out, wo, bias_o).astype(np.float32)


def kernel(_trace=False, **inputs):
    causal, in_maps = _prep(**inputs)
    if not causal:
        return _kernel_numpy(**inputs)
    nc = _get(causal)
    res = run_bass_kernel_spmd(nc, in_maps, list(range(NC)), trace=_trace)
    y = np.empty((B, L, D), np.float32)
    for c in range(NC):
        y[:, LW * c:LW * (c + 1), :] = res.results[c]["yts"].transpose(0, 2, 1)
    if _trace:
        kernel._last = res
    return y


def bench(inputs, repeats=(1, 5), iters=5):
    """Kept for API compat; paired-repeat timing is unreliable under axon."""
    return -1.0, {}
